# revision 36
# baseline (speedup 1.0000x reference)
"""Trainium2 Bass kernel for nn_KANNeuron (Chebyshev-KAN neuron layer).

Math: out[b] = hw * sum_d sum_k c[d,k] * T_k(tanh(x[b,d]))
Half-degree u-basis (u = 2t^2-1 = T_2(t), t = tanh(x)) keeps every
stream bounded in [-1,1] and coefficients O(0.4):
    out[b] = hw * ( C0 + sum_d [ sum_{m=1..4} ae[d,m] u^m
                               + sum_{m=0..3} ao[d,m] t u^m ] )
Eight mover streams per dim group, reduced over dims by the PE with
per-dim stationary coefficient columns:
    fp16: t (ao0), u (ae1), u2 (ae2)       [error-sensitive, 1 col/cyc]
    bf16: u3 (ae3), u4 (ae4), v1=t*u (ao1), v2=t*u2 (ao2),
          v3=t*u3 (ao3)                    [double-pumped, 2 col/cyc]
Engine split per group-chunk [128 dims, 2048 batch]:
    scalar: tanh, q=t^2 (Square), u4 (Square)
    vector: u = 2q-1 (tensor_scalar 4x), u2, u3, v1, v2 (TT 2x)
    gpsimd: v3
    PE: 32 matmuls into a persistent PSUM [1, 2048] fp32 accumulator
Host adds C0 (batch-independent constant) and horizontal_weight.

Device strategy: 8 cores, batch-sharded 2048 rows each; host
pre-transposes x so partitions = dims-in-group, free = batch.
"""

import os
from functools import lru_cache

import numpy as np

import concourse.bass as bass
import concourse.tile as tile
from concourse import mybir
from concourse.bass_utils import run_bass_kernel_spmd
from concourse.vector_clock import ScopedClock, VectorClock

# ---------------------------------------------------------------- constants
B = 16384
D = 1024
DEG = 8
NCORES = 8
BPC = B // NCORES        # 2048 batch rows per core
P = 128                  # partitions
G = D // P               # 8 dim groups
C = 512                  # psum bank width (fp32)
NBLK = BPC // C          # 4 psum column blocks

F16 = mybir.dt.float16
BF16 = mybir.dt.bfloat16
F32 = mybir.dt.float32

LAST_EXEC_NS = None      # filled when KERNEL_PROFILE=1
LAST_PROFILE = None


# ------------------------------------------------------- walrus workaround
def _split_drain_and_barrier(self, tick_clock, wait_clock):
    """Tile's final Drain carries one sem-wait per ticked logical processor;
    walrus CoreV2/V3 codegen rejects >1 sync wait on a CTRL instruction.
    Split the waits across single-wait NOPs on the sync engine instead."""
    gc = tick_clock.global_clock
    vals = list(gc)
    for i, v in enumerate(vals):
        if v <= 0:
            continue
        sub = [0] * len(vals)
        sub[i] = v
        nop = self.nc.sync.nop(nofuse=True, hint="drain_split_wait")
        wait_clock.add_sem_waits(nop.ins, ScopedClock({None: VectorClock(sub)}))
    self.nc.sync.drain()
    self.nc.all_engine_barrier()
    assert self.sems is not None
    popped = self.nc._tile_sem_poison_stack.pop()
    assert popped is self._sem_poison
    self.nc.clear_and_free_semaphores(list(self.sems.allocated().values()))
    self.nc.all_engine_barrier()


tile.TileContext._drain_and_barrier = _split_drain_and_barrier

# This container's walrus (CoreV2/V3 codegen) supports at most ONE sync wait
# per instruction. Tile routinely attaches several. Hoist all but the last
# wait of every committed instruction onto same-engine no-fuse NOPs placed
# immediately before it (engine programs are sequential, so semantics hold).
_orig_commit_instruction = tile.TileContext._commit_instruction
_wsplit_seq = [0]


def _commit_split_waits(self, inst, lazy_reg_writes=True):
    si = getattr(inst, "sync_info", None)
    eng = getattr(inst, "engine", None)
    if (
        si is not None
        and si.on_wait is not None
        and len(si.on_wait) > 1
        and eng is not None
    ):
        waits = list(si.on_wait)
        for w in waits[:-1]:
            _wsplit_seq[0] += 1
            nop = mybir.InstNoOp(
                name=f"wsplit_{_wsplit_seq[0]}",
                engine=eng,
                bass_nofuse=True,
                sync_info=mybir.SyncInfo(on_wait=[w], on_update=[]),
            )
            _orig_commit_instruction(self, nop, lazy_reg_writes=False)
        inst.sync_info = mybir.SyncInfo(
            on_wait=[waits[-1]], on_update=list(si.on_update or [])
        )
    return _orig_commit_instruction(self, inst, lazy_reg_writes)


tile.TileContext._commit_instruction = _commit_split_waits


# --------------------------------------------- NTFF profiling hook (axon)
def _install_ntff_hook():
    """This container's trn_rl_repo lacks antenv.axon_hooks; recreate the
    ctypes NTFF hook against the loaded libaxon_pjrt.so so trace=True works."""
    import contextlib
    import ctypes
    import sys
    import types

    try:
        from antenv.axon_hooks import get_axon_ntff_profile_hook  # noqa: F401

        return
    except ImportError:
        pass

    so_path = os.environ.get("AXON_PJRT_SO", "/opt/axon/libaxon_pjrt.so")
    hook = None
    if os.path.exists(so_path):
        lib = ctypes.CDLL(so_path)
        if hasattr(lib, "axon_start_nrt_profile"):
            lib.axon_start_nrt_profile.argtypes = [
                ctypes.POINTER(ctypes.c_int64),
                ctypes.c_size_t,
            ]
            lib.axon_start_nrt_profile.restype = ctypes.c_int64
            lib.axon_stop_nrt_profile.argtypes = [ctypes.c_char_p]
            lib.axon_stop_nrt_profile.restype = ctypes.c_int64

            @contextlib.contextmanager
            def _hook(output_dir, device_ids):
                import jax

                jax.devices()
                if device_ids:
                    ids = (ctypes.c_int64 * len(device_ids))(*device_ids)
                    rc = lib.axon_start_nrt_profile(ids, len(device_ids))
                else:
                    rc = lib.axon_start_nrt_profile(None, 0)
                if rc != 0:
                    raise RuntimeError(f"axon_start_nrt_profile rc={rc}")
                try:
                    yield
                finally:
                    n = lib.axon_stop_nrt_profile(str(output_dir).encode())
                    if n < 0:
                        raise RuntimeError(f"axon_stop_nrt_profile rc={n}")

            hook = _hook

    mod = types.ModuleType("antenv.axon_hooks")
    mod.get_axon_ntff_profile_hook = lambda: hook
    mod.set_axon_ntff_profile_hook = lambda h: None
    sys.modules["antenv.axon_hooks"] = mod


_install_ntff_hook()


# Artifact upload needs bucket creds this container may not have; degrade.
import concourse.bass_utils as _bu  # noqa: E402

_orig_upload_artifacts = _bu.upload_artifacts


def _safe_upload_artifacts(tmpdir):
    try:
        return _orig_upload_artifacts(tmpdir)
    except Exception:
        return str(tmpdir)


_bu.upload_artifacts = _safe_upload_artifacts


# ------------------------------------------------------------- host helpers
def _cheb_to_monomial_matrix(deg: int) -> np.ndarray:
    """M[k, j]: T_k(u) = sum_j M[k, j] u^j (float64, exact integers)."""
    M = np.zeros((deg + 1, deg + 1))
    M[0, 0] = 1.0
    if deg >= 1:
        M[1, 1] = 1.0
    for k in range(2, deg + 1):
        M[k, 1:] += 2.0 * M[k - 1, :-1]
        M[k, :] -= M[k - 2, :]
    return M


def _odd_r_matrix() -> np.ndarray:
    """R[m, j]: T_{2m+1}(t) = t * sum_j R[m, j] u^j with u = 2t^2-1."""
    R = np.zeros((4, 4))
    R[0, 0] = 1.0
    R[1, 0], R[1, 1] = -1.0, 2.0
    for m in range(2, 4):
        R[m, 1:] += 2.0 * R[m - 1, :-1]
        R[m, :] -= R[m - 2, :]
    return R


# ------------------------------------------------------------ device program
@lru_cache(maxsize=1)
def _build_program():
    v3_eng = os.environ.get("KAN_V3_ENGINE", "vector")
    nc = bass.Bass(trn_type="TRN2", target_bir_lowering=False, num_devices=NCORES)
    xp_ext = nc.dram_tensor("xp", [P, G * BPC], F16, kind="ExternalInput").ap()
    # fp16 stationary: t, u, u2 per group
    wsf_ext = nc.dram_tensor("wsf", [P, G * 3], F16, kind="ExternalInput").ap()
    # bf16 stationary: u3, u4, v1, v2, v3 per group
    wsb_ext = nc.dram_tensor("wsb", [P, G * 5], BF16, kind="ExternalInput").ap()
    out_ext = nc.dram_tensor("out", [1, BPC], F32, kind="ExternalOutput").ap()

    with tile.TileContext(nc) as tc:
        with (
            tc.tile_pool(name="singles", bufs=1) as singles,
            tc.tile_pool(name="psp", bufs=1, space="PSUM") as psp,
            tc.tile_pool(name="xin", bufs=4) as xin,
            tc.tile_pool(name="pw", bufs=4) as pw,
            tc.tile_pool(name="osb", bufs=1) as osb,
        ):
            # col-tiled PE: batch block j accumulates at psum partition 32*j
            # (tile_position=(0,32j) runs the 4 blocks concurrently on
            # disjoint 32-column bands of the PE array)
            ps = psp.tile([128, BPC], F32)

            # prefetch all x chunks up-front across two DMA queues;
            # weights ride the gpsimd queue after the first x chunks
            xts = []
            dma_engs = [nc.sync, nc.gpsimd]
            H = BPC // 2
            for g in range(G):
                xt = xin.tile([P, BPC], F16, tag="x")
                if g < 2:
                    # split the lead-in chunks across both queues so the
                    # first tanh can start ~2us earlier
                    nc.sync.dma_start(
                        out=xt[:, 0:H], in_=xp_ext[:, g * BPC : g * BPC + H]
                    )
                    nc.gpsimd.dma_start(
                        out=xt[:, H:BPC], in_=xp_ext[:, g * BPC + H : (g + 1) * BPC]
                    )
                else:
                    dma_engs[g % 2].dma_start(
                        out=xt[:], in_=xp_ext[:, g * BPC : (g + 1) * BPC]
                    )
                xts.append(xt)
            wsf = singles.tile([P, G * 3], F16)
            nc.gpsimd.dma_start(out=wsf[:], in_=wsf_ext[:, :])
            wsb = singles.tile([P, G * 5], BF16)
            nc.gpsimd.dma_start(out=wsb[:], in_=wsb_ext[:, :])

            # two-chunk software pipelining: interleave ops of chunk pairs so
            # no engine instruction directly follows the op it depends on
            # (hides ~2us semaphore latency per dependent link)
            SCALAR_U4 = 256  # u4 columns done on scalar; rest on vector
            for half in range(G // 2):
                cs = [2 * half, 2 * half + 1]
                tl, ql, ul, u2l, u3l, u4l, v1l, v2l, v3l = ({} for _ in range(9))
                for c in cs:
                    tl[c] = pw.tile([P, BPC], F16, tag="t", name=f"t{c}")
                    nc.scalar.activation(
                        tl[c][:], xts[c][:], mybir.ActivationFunctionType.Tanh
                    )
                for c in cs:
                    ql[c] = pw.tile([P, BPC], F16, tag="q", name=f"q{c}")
                    nc.scalar.square(ql[c][:], tl[c][:])
                for c in cs:
                    ul[c] = pw.tile([P, BPC], F16, tag="u", name=f"u{c}")
                    nc.vector.tensor_scalar(
                        ul[c][:], ql[c][:], 2.0, -1.0,
                        mybir.AluOpType.mult, mybir.AluOpType.add,
                    )
                for c in cs:
                    u2l[c] = pw.tile([P, BPC], F16, tag="u2", name=f"u2_{c}")
                    nc.scalar.square(u2l[c][:], ul[c][:])
                for c in cs:
                    v1l[c] = pw.tile([P, BPC], BF16, tag="v1", name=f"v1_{c}")
                    nc.vector.tensor_mul(v1l[c][:], tl[c][:], ul[c][:])
                for c in cs:
                    u3l[c] = pw.tile([P, BPC], BF16, tag="u3", name=f"u3_{c}")
                    nc.vector.tensor_mul(u3l[c][:], ul[c][:], u2l[c][:])
                for c in cs:
                    u4l[c] = pw.tile([P, BPC], BF16, tag="u4", name=f"u4_{c}")
                    nc.scalar.square(u4l[c][:, 0:SCALAR_U4], u2l[c][:, 0:SCALAR_U4])
                    nc.vector.tensor_mul(
                        u4l[c][:, SCALAR_U4:BPC],
                        u2l[c][:, SCALAR_U4:BPC],
                        u2l[c][:, SCALAR_U4:BPC],
                    )
                for c in cs:
                    v2l[c] = pw.tile([P, BPC], BF16, tag="v2", name=f"v2_{c}")
                    nc.vector.tensor_mul(v2l[c][:], tl[c][:], u2l[c][:])
                for c in cs:
                    v3l[c] = pw.tile([P, BPC], BF16, tag="v3", name=f"v3_{c}")
                    nc.vector.tensor_mul(v3l[c][:], tl[c][:], u3l[c][:])
                for c in cs:
                    movers = [
                        (tl[c], wsf, c * 3 + 0),
                        (ul[c], wsf, c * 3 + 1),
                        (v1l[c], wsb, c * 5 + 2),
                        (u2l[c], wsf, c * 3 + 2),
                        (u3l[c], wsb, c * 5 + 0),
                        (u4l[c], wsb, c * 5 + 1),
                        (v2l[c], wsb, c * 5 + 3),
                        (v3l[c], wsb, c * 5 + 4),
                    ]
                    for s, (mov, wtile, col) in enumerate(movers):
                        for j in range(NBLK):
                            nc.tensor.matmul(
                                ps[32 * j : 32 * j + 1, j * C : (j + 1) * C],
                                wtile[:, col : col + 1],
                                mov[:, j * C : (j + 1) * C],
                                start=(c == 0 and s == 0),
                                stop=(c == G - 1 and s == len(movers) - 1),
                                tile_position=(0, 32 * j),
                            )

            ob = osb.tile([1, BPC], F32)
            for j in range(NBLK):
                src = ps[32 * j : 32 * j + 1, j * C : (j + 1) * C]
                dst = ob[0:1, j * C : (j + 1) * C]
                if j % 2 == 0:
                    nc.scalar.copy(dst, src)
                else:
                    nc.vector.tensor_copy(dst, src)
            nc.sync.dma_start(out=out_ext[0:1, :], in_=ob[:])

    return nc


# ------------------------------------------------------------------- kernel
def kernel(x, coefficients, horizontal_weight, degree):
    global LAST_EXEC_NS, LAST_PROFILE
    x = np.asarray(x, dtype=np.float32)
    coefficients = np.asarray(coefficients, dtype=np.float32)
    hw = float(np.asarray(horizontal_weight).reshape(-1)[0])
    deg = int(np.asarray(degree))
    assert deg == DEG and x.shape == (B, D) and coefficients.shape == (D * (DEG + 1),)

    cm = coefficients.astype(np.float64).reshape(D, DEG + 1)
    ae = cm[:, 0::2] @ _cheb_to_monomial_matrix(4)   # [D, 5] coeffs of u^m
    ao = cm[:, 1::2] @ _odd_r_matrix()               # [D, 4] coeffs of t*u^m
    C0_total = float(ae[:, 0].sum())

    import ml_dtypes

    def to_pg(col):  # [D] -> [P, G]
        return np.ascontiguousarray(col.reshape(G, P).T)

    # fp16 stationary: t -> ao0, u -> ae1, u2 -> ae2
    wsf_np = np.stack(
        [to_pg(ao[:, 0]), to_pg(ae[:, 1]), to_pg(ae[:, 2])], axis=2
    ).reshape(P, G * 3).astype(np.float16)
    # bf16 stationary: u3 -> ae3, u4 -> ae4, v1 -> ao1, v2 -> ao2, v3 -> ao3
    wsb_np = np.stack(
        [to_pg(ae[:, 3]), to_pg(ae[:, 4]), to_pg(ao[:, 1]),
         to_pg(ao[:, 2]), to_pg(ao[:, 3])], axis=2
    ).reshape(P, G * 5).astype(ml_dtypes.bfloat16)

    in_maps = []
    for core in range(NCORES):
        xc = x[core * BPC : (core + 1) * BPC, :]  # [BPC, D]
        # xp[p, g*BPC + b] = x[core*BPC + b, g*P + p]
        xp = (
            xc.reshape(BPC, G, P).transpose(2, 1, 0).reshape(P, G * BPC)
        ).astype(np.float16)
        in_maps.append({"xp": xp, "wsf": wsf_np, "wsb": wsb_np})

    nc = _build_program()
    trace = os.environ.get("KERNEL_PROFILE") == "1"
    res = run_bass_kernel_spmd(nc, in_maps, list(range(NCORES)), trace=trace)
    if trace:
        LAST_EXEC_NS = res.exec_time_ns
        LAST_PROFILE = res.profile_json

    out = np.empty(B, dtype=np.float32)
    for core in range(NCORES):
        out[core * BPC : (core + 1) * BPC] = res.results[core]["out"].reshape(BPC)
    return ((out + C0_total) * hw).astype(np.float32)
